# revision 1
# baseline (speedup 1.0000x reference)
"""MoE ExpertGroup kernel for Trainium2 (8 NeuronCores, expert-parallel).

Problem: E=8 experts, H=1024, I=4096, N=16384 tokens sorted by expert.
y[t] = gelu_tanh(x[t] @ w1[e(t)]) @ w2[e(t)]

Sharding: expert-parallel — core e holds expert e's weights and processes
expert e's token block (balanced routing: 2048 tokens/core). Host ships
each core's tokens transposed (xT [H,T]) in bf16; device returns y [T,H]
fp32; host scatters rows back. All matmul operands are bf16 (full-rate on
the PE like fp32r, half the DMA traffic; rel err ~4e-3 vs 2e-2 budget).

Per-core structure (PE floor 2048 matmuls x 512 cols = 437us @2.4GHz):
- x fully resident in SBUF; w2 fully resident (loaded once); w1 streamed
  per 4-I-tile group, double buffered.
- Tokens in 4 blocks of 512; MM1 (hT = gelu(w1.T @ xT)) runs on block
  pairs so each w1 stationary tile covers 1024 moving columns.
- MM2 (y = hT.T @ w2) accumulates all 32 I-tiles of a token tile in one
  PSUM group: no cross-group accumulation work at all.
- DMA issue order tracks PE consumption order (descriptors execute
  in order across the striped engines): first w1 group interleaved per
  k-slice with x(b0+b1) as single 2KB-line transfers (adjacent columns
  merged - better DMA line efficiency), w2 chunks spread across MM1's
  g-loop.
- 15 warmup matmuls ramp the PE clock while the first DMAs land.
"""

import sys

sys.path.insert(0, "/opt/trn_rl_repo")

import numpy as np
import ml_dtypes

E = 8
H = 1024
I = 4096
N_TOK = 16384
T = N_TOK // E

P = 128
TB = 512
NB = T // TB          # 4 token blocks
HB = H // P           # 8
IB = I // P           # 32
GI = 4                # i-tiles per w1 DMA group
NG = IB // GI         # 8

_CACHE = {}


def _build():
    import concourse.bacc as bacc
    import concourse.mybir as mybir
    import concourse.tile as tile

    F32 = mybir.dt.float32
    BF16 = mybir.dt.bfloat16
    GELU = mybir.ActivationFunctionType.Gelu_apprx_tanh
    COPY = mybir.ActivationFunctionType.Copy

    nc = bacc.Bacc("TRN2", target_bir_lowering=False, debug=False, num_devices=E)

    xT = nc.dram_tensor("xT", [H, T], BF16, kind="ExternalInput").ap()
    w1 = nc.dram_tensor("w1", [H, I], BF16, kind="ExternalInput").ap()
    w2 = nc.dram_tensor("w2", [I, H], BF16, kind="ExternalInput").ap()
    y = nc.dram_tensor("y", [T, H], F32, kind="ExternalOutput").ap()

    with tile.TileContext(nc) as tc:
        with (
            tc.tile_pool(name="xp", bufs=1) as x_pool,
            tc.tile_pool(name="w2p", bufs=1) as w2_pool,
            tc.tile_pool(name="w1p", bufs=2) as w1_pool,
            tc.tile_pool(name="hp", bufs=1) as h_pool,
            tc.tile_pool(name="yp", bufs=4) as y_pool,
            tc.tile_pool(name="ph", bufs=4, space="PSUM") as ph_pool,
            tc.tile_pool(name="pyA", bufs=2, space="PSUM") as pyA_pool,
            tc.tile_pool(name="pyB", bufs=2, space="PSUM") as pyB_pool,
        ):
            # warmup first: memset on gpsimd runs while DMAs start flowing
            warm = y_pool.tile([P, TB], BF16, tag="warm", name="warm")
            nc.vector.memset(warm[:], 0.0)
            for wi in range(15):
                pw = ph_pool.tile([P, TB], F32, tag="ph", name="pw")
                nc.tensor.matmul(pw[:], warm[:, :P], warm[:], start=True, stop=True)

            xt = [x_pool.tile([P, T], BF16, tag=f"x{k}", name=f"x{k}") for k in range(HB)]

            def load_x(b):
                for k in range(HB):
                    nc.sync.dma_start(
                        out=xt[k][:, b * TB : (b + 1) * TB],
                        in_=xT[k * P : (k + 1) * P, b * TB : (b + 1) * TB],
                    )

            def load_w1(g):
                tiles = []
                for k in range(HB):
                    wt = w1_pool.tile([P, GI * P], BF16, tag=f"w1_{k}", name=f"w1_{k}")
                    nc.sync.dma_start(
                        out=wt[:],
                        in_=w1[k * P : (k + 1) * P, g * GI * P : (g + 1) * GI * P],
                    )
                    tiles.append(wt)
                return tiles

            w2t = [None] * IB

            def load_w2(i0, n):
                for i in range(i0, min(i0 + n, IB)):
                    wt = w2_pool.tile([P, H], BF16, tag=f"w2_{i}", name=f"w2_{i}")
                    nc.sync.dma_start(out=wt[:], in_=w2[i * P : (i + 1) * P, :])
                    w2t[i] = wt

            hT = {}

            def mm1_group(b0, g, w1t):
                """MM1 + gelu for token blocks (b0, b0+1), i-group g."""
                for il in range(GI):
                    i = g * GI + il
                    ph = [
                        ph_pool.tile([P, TB], F32, tag="ph", name="ph")
                        for _ in range(2)
                    ]
                    for k in range(HB):
                        for j in range(2):
                            ts_ = slice((b0 + j) * TB, (b0 + j + 1) * TB)
                            nc.tensor.matmul(
                                ph[j][:],
                                w1t[k][:, il * P : (il + 1) * P],
                                xt[k][:, ts_],
                                start=(k == 0),
                                stop=(k == HB - 1),
                            )
                    for j in range(2):
                        st = (b0 + j) % 2
                        ht = h_pool.tile(
                            [P, TB], BF16, tag=f"h{st}_{i}", name=f"h{st}_{i}"
                        )
                        nc.scalar.activation(ht[:], ph[j][:], GELU)
                        hT[(st, i)] = ht

            def mm2_block(b):
                st = b % 2
                for tc_ in range(TB // P):
                    pa = pyA_pool.tile([P, H // 2], F32, tag="pyA", name="pyA")
                    pb = pyB_pool.tile([P, H // 2], F32, tag="pyB", name="pyB")
                    for i in range(IB):
                        hs = hT[(st, i)][:, tc_ * P : (tc_ + 1) * P]
                        nc.tensor.matmul(
                            pa[:], hs, w2t[i][:, : H // 2],
                            start=(i == 0), stop=(i == IB - 1),
                        )
                        nc.tensor.matmul(
                            pb[:], hs, w2t[i][:, H // 2 :],
                            start=(i == 0), stop=(i == IB - 1),
                        )
                    ysb = y_pool.tile([P, H], F32, tag="yt", name="yt")
                    t0 = b * TB + tc_ * P
                    nc.scalar.activation(ysb[:, : H // 2], pa[:], COPY)
                    nc.sync.dma_start(
                        out=y[t0 : t0 + P, : H // 2], in_=ysb[:, : H // 2]
                    )
                    nc.scalar.activation(ysb[:, H // 2 :], pb[:], COPY)
                    nc.sync.dma_start(
                        out=y[t0 : t0 + P, H // 2 :], in_=ysb[:, H // 2 :]
                    )

            # ---- pair (b0,b1): interleave w1(g0)/x(b0)/x(b1) per k-tile so
            # the first MM1 chains start as soon as each k-slice lands
            w1t_cur = []
            for k in range(HB):
                wt = w1_pool.tile([P, GI * P], BF16, tag=f"w1_{k}", name=f"w1_{k}")
                nc.sync.dma_start(out=wt[:], in_=w1[k * P : (k + 1) * P, 0 : GI * P])
                w1t_cur.append(wt)
                # b0+b1 are adjacent columns: one 2KB-line transfer per k
                nc.sync.dma_start(
                    out=xt[k][:, 0 : 2 * TB],
                    in_=xT[k * P : (k + 1) * P, 0 : 2 * TB],
                )
            load_w2(0, 4)
            for g in range(NG):
                w1t_next = load_w1(g + 1) if g + 1 < NG else None
                load_w2(4 * (g + 1), 4)
                if g == 5:
                    for k in range(HB):
                        nc.sync.dma_start(
                            out=xt[k][:, 2 * TB : 4 * TB],
                            in_=xT[k * P : (k + 1) * P, 2 * TB : 4 * TB],
                        )
                mm1_group(0, g, w1t_cur)
                w1t_cur = w1t_next

            # prefetch first two w1 groups for pair (b2,b3) before the y DMAs
            # of mm2 enter the queue
            w1t_p2 = [load_w1(0), load_w1(1)]

            mm2_block(0)
            mm2_block(1)

            for g in range(NG):
                if g + 2 < NG:
                    w1t_p2.append(load_w1(g + 2))
                mm1_group(2, g, w1t_p2[g])

            mm2_block(2)
            mm2_block(3)

    nc.compile()
    return nc


def _get_nc():
    if "nc" not in _CACHE:
        _CACHE["nc"] = _build()
    return _CACHE["nc"]


def _prep_in_maps(x_sorted, w1, w2, expert_counts):
    counts = np.asarray(expert_counts, dtype=np.int64)
    n = x_sorted.shape[0]
    offsets = np.cumsum(counts)
    eid = np.searchsorted(offsets, np.arange(n), side="right")

    in_maps = []
    row_idx = []
    for e in range(E):
        rows = np.nonzero(eid == e)[0]
        assert len(rows) <= T, f"expert {e} overflows capacity {T}"
        xe = np.zeros((T, H), dtype=np.float32)
        xe[: len(rows)] = x_sorted[rows]
        row_idx.append(rows)
        in_maps.append(
            {
                "xT": np.ascontiguousarray(xe.T).astype(ml_dtypes.bfloat16),
                "w1": np.asarray(w1[e]).astype(ml_dtypes.bfloat16),
                "w2": np.asarray(w2[e]).astype(ml_dtypes.bfloat16),
            }
        )
    return in_maps, row_idx


def kernel(x_sorted, w1, w2, expert_counts, local_expert_indices, **_unused):
    from concourse.bass_utils import run_bass_kernel_spmd

    x_sorted = np.ascontiguousarray(x_sorted, dtype=np.float32)
    nc = _get_nc()
    in_maps, row_idx = _prep_in_maps(x_sorted, w1, w2, expert_counts)
    res = run_bass_kernel_spmd(nc, in_maps, list(range(E))).results

    n = x_sorted.shape[0]
    out = np.zeros((n, H), dtype=np.float32)
    for e in range(E):
        rows = row_idx[e]
        out[rows] = np.asarray(res[e]["y"][: len(rows)], dtype=np.float32)
    return out



# revision 2
# speedup vs baseline: 1.0017x; 1.0017x over previous
"""MoE ExpertGroup kernel for Trainium2 (8 NeuronCores, expert-parallel).

Problem: E=8 experts, H=1024, I=4096, N=16384 tokens sorted by expert.
y[t] = gelu_tanh(x[t] @ w1[e(t)]) @ w2[e(t)]

Sharding: expert-parallel - core e holds expert e's weights and processes
expert e's token block (balanced routing: 2048 tokens/core). All matmul
operands are bf16 (full-rate on the PE, half the DMA traffic); y is
returned in bf16 and upcast on the host (rel err ~4e-3 vs 2e-2 budget).

Per-core structure (PE stream floor: 2048 matmuls x 512 cols @2.4GHz
= 442us; everything else is edges):
- Host pre-packs x/w1/w2 into k-major SBUF-shaped layouts so each DMA
  is ONE large contiguous-line transfer (the Sync queue issues triggers
  at only ~1.6/us, and each trigger fans out over all 16 DMA engines).
- MM1 pass 1 (blocks b0,b1): group g0 runs single-block chains (all b0,
  then all b1) with per-i-tile w1 triggers, so the first chain needs
  only 1.25MB landed (~12us) instead of 3MB (~18us). Groups g1..g7 run
  b0+b1 chains off one 1MB group tile, double buffered, prefetched one
  group ahead.
- MM2 accumulates all 32 I-tiles of a token tile into one PSUM group.
  y stored as bf16. For the very last token tile the second output half
  is drained by the Vector engine in parallel with the Scalar act to
  shorten the tail.
- 12 warmup matmuls ramp the PE clock (HAM un-throttle needs ~3.4us of
  busy) while the first DMAs land.
"""

import sys

sys.path.insert(0, "/opt/trn_rl_repo")

import numpy as np
import ml_dtypes

E = 8
H = 1024
I = 4096
N_TOK = 16384
T = N_TOK // E

P = 128
TB = 512
NB = T // TB          # 4 token blocks
HB = H // P           # 8 k-tiles (contraction for MM1)
IB = I // P           # 32 i-tiles
GI = 4                # i-tiles per w1 group
NG = IB // GI         # 8 groups
WARM = 12

_CACHE = {}


def _build():
    import concourse.bacc as bacc
    import concourse.mybir as mybir
    import concourse.tile as tile

    F32 = mybir.dt.float32
    BF16 = mybir.dt.bfloat16
    GELU = mybir.ActivationFunctionType.Gelu_apprx_tanh
    COPY = mybir.ActivationFunctionType.Copy

    nc = bacc.Bacc("TRN2", target_bir_lowering=False, debug=False, num_devices=E)

    # Host-packed layouts (see _prep_in_maps):
    #   xg [P, (b,k,c)]   : xg[p, b*HB*TB + k*TB + c] = x[b*TB+c, k*P+p]
    #   w1a [P, (il,k,c)] : w1a[p, il*HB*P + k*P + c] = w1[k*P+p, il*P+c]
    #   w1b [P, (g',k,c)] : w1b[p, g'*HB*TB + k*TB + c] = w1[k*P+p, (g'+1)*TB+c]
    #   w2h [P, (i,c)]    : w2h[p, i*H + c] = w2[i*P+p, c]
    xg = nc.dram_tensor("xg", [P, NB * HB * TB], BF16, kind="ExternalInput").ap()
    w1a = nc.dram_tensor("w1a", [P, GI * HB * P], BF16, kind="ExternalInput").ap()
    w1b = nc.dram_tensor(
        "w1b", [P, (NG - 1) * HB * TB], BF16, kind="ExternalInput"
    ).ap()
    w2h = nc.dram_tensor("w2h", [P, IB * H], BF16, kind="ExternalInput").ap()
    y = nc.dram_tensor("y", [T, H], BF16, kind="ExternalOutput").ap()

    with tile.TileContext(nc) as tc:
        with (
            tc.tile_pool(name="xp", bufs=1) as x_pool,
            tc.tile_pool(name="w1ap", bufs=1) as w1a_pool,
            tc.tile_pool(name="w1p", bufs=2) as w1_pool,
            tc.tile_pool(name="w2p", bufs=1) as w2_pool,
            tc.tile_pool(name="hp", bufs=1) as h_pool,
            tc.tile_pool(name="yp", bufs=4) as y_pool,
            tc.tile_pool(name="ph", bufs=4, space="PSUM") as ph_pool,
            tc.tile_pool(name="pyA", bufs=2, space="PSUM") as pyA_pool,
            tc.tile_pool(name="pyB", bufs=2, space="PSUM") as pyB_pool,
        ):
            # warmup first: PE busy while the first DMAs land (HAM ramp)
            warm = y_pool.tile([P, TB], BF16, tag="warm", name="warm")
            nc.vector.memset(warm[:], 0.0)
            for wi in range(WARM):
                pw = ph_pool.tile([P, TB], F32, tag="ph", name="pw")
                nc.tensor.matmul(pw[:], warm[:, :P], warm[:], start=True, stop=True)

            # resident tiles
            xt = [
                x_pool.tile([P, HB * TB], BF16, tag=f"x{b}", name=f"x{b}")
                for b in range(NB)
            ]
            w1at = [
                w1a_pool.tile([P, HB * P], BF16, tag=f"a{il}", name=f"a{il}")
                for il in range(GI)
            ]
            w2t = [
                w2_pool.tile([P, GI * H], BF16, tag=f"q{q}", name=f"q{q}")
                for q in range(IB // GI)
            ]

            def trig_x(b):
                nc.sync.dma_start(
                    out=xt[b][:], in_=xg[:, b * HB * TB : (b + 1) * HB * TB]
                )

            def trig_w1a(il):
                nc.sync.dma_start(
                    out=w1at[il][:], in_=w1a[:, il * HB * P : (il + 1) * HB * P]
                )

            def trig_w1g(g):
                wt = w1_pool.tile([P, HB * TB], BF16, tag="wg", name="wg")
                nc.sync.dma_start(
                    out=wt[:], in_=w1b[:, (g - 1) * HB * TB : g * HB * TB]
                )
                return wt

            def trig_w1g0b():
                wt = w1a_pool.tile([P, GI * HB * P], BF16, tag="g0b", name="g0b")
                nc.sync.dma_start(out=wt[:], in_=w1a[:])
                return wt

            def trig_w2(q):
                nc.sync.dma_start(
                    out=w2t[q][:], in_=w2h[:, q * GI * H : (q + 1) * GI * H]
                )

            hT = {}

            def chain(b, i, lhsT_of_k):
                """One MM1 chain: hT[b%2, i] = gelu(w1[:, i-tile].T @ x[b])."""
                pht = ph_pool.tile([P, TB], F32, tag="ph", name="ph")
                for k in range(HB):
                    nc.tensor.matmul(
                        pht[:],
                        lhsT_of_k(k),
                        xt[b][:, k * TB : (k + 1) * TB],
                        start=(k == 0),
                        stop=(k == HB - 1),
                    )
                st = b % 2
                ht = h_pool.tile([P, TB], BF16, tag=f"h{st}_{i}", name=f"h{st}_{i}")
                nc.scalar.activation(ht[:], pht[:], GELU)
                hT[(st, i)] = ht

            def mm2_block(b, tail=False):
                st = b % 2
                for tc_ in range(TB // P):
                    pa = pyA_pool.tile([P, TB], F32, tag="pyA", name="pyA")
                    pb = pyB_pool.tile([P, TB], F32, tag="pyB", name="pyB")
                    for i in range(IB):
                        hs = hT[(st, i)][:, tc_ * P : (tc_ + 1) * P]
                        w2v = w2t[i // GI][:, (i % GI) * H : (i % GI + 1) * H]
                        nc.tensor.matmul(
                            pa[:], hs, w2v[:, : H // 2],
                            start=(i == 0), stop=(i == IB - 1),
                        )
                        nc.tensor.matmul(
                            pb[:], hs, w2v[:, H // 2 :],
                            start=(i == 0), stop=(i == IB - 1),
                        )
                    ysb = y_pool.tile([P, H], BF16, tag="yt", name="yt")
                    t0 = b * TB + tc_ * P
                    nc.scalar.activation(ysb[:, : H // 2], pa[:], COPY)
                    nc.sync.dma_start(
                        out=y[t0 : t0 + P, : H // 2], in_=ysb[:, : H // 2]
                    )
                    if tail and tc_ == TB // P - 1:
                        # final tile: drain the second half on the Vector
                        # engine (different PSUM bank than pa) in parallel
                        nc.vector.tensor_copy(ysb[:, H // 2 :], pb[:])
                    else:
                        nc.scalar.activation(ysb[:, H // 2 :], pb[:], COPY)
                    nc.sync.dma_start(
                        out=y[t0 : t0 + P, H // 2 :], in_=ysb[:, H // 2 :]
                    )

            # ---- prologue DMA triggers (order = landing order) ----
            trig_w1a(0)
            trig_x(0)
            trig_w1a(1)
            trig_w1a(2)
            trig_w1a(3)
            trig_x(1)
            w1t_cur = trig_w1g(1)

            # ---- pass 1 (blocks 0,1) ----
            # g0: single-block chains off the per-i-tile w1a tiles
            for b in (0, 1):
                for il in range(GI):
                    chain(b, il, lambda k, il=il: w1at[il][:, k * P : (k + 1) * P])
            # g1..g7: paired blocks off 1MB group tiles, prefetch g+1
            for g in range(1, NG):
                w1t_next = trig_w1g(g + 1) if g + 1 < NG else None
                if g == 2:
                    trig_w2(0)
                elif g == 3:
                    trig_w2(1)
                elif g == 4:
                    trig_w2(2)
                    trig_x(2)
                elif g == 5:
                    trig_w2(3)
                    trig_x(3)
                elif g == 6:
                    trig_w2(4)
                elif g == 7:
                    trig_w2(5)
                for b in (0, 1):
                    for il in range(GI):
                        i = g * GI + il
                        chain(
                            b,
                            i,
                            lambda k, il=il: w1t_cur[
                                :, k * TB + il * P : k * TB + (il + 1) * P
                            ],
                        )
                w1t_cur = w1t_next

            # prefetch pass-2 w1 (g0 packed tile + group 1) before the y DMAs
            # of mm2 enter the Sync queue
            w1g0b = trig_w1g0b()
            w1t_cur = trig_w1g(1)
            trig_w2(6)
            trig_w2(7)

            mm2_block(0)
            mm2_block(1)

            # ---- pass 2 (blocks 2,3) ----
            for b in (2, 3):
                for il in range(GI):
                    chain(
                        b,
                        il,
                        lambda k, il=il: w1g0b[
                            :, il * HB * P + k * P : il * HB * P + (k + 1) * P
                        ],
                    )
            for g in range(1, NG):
                w1t_next = trig_w1g(g + 1) if g + 1 < NG else None
                for b in (2, 3):
                    for il in range(GI):
                        i = g * GI + il
                        chain(
                            b,
                            i,
                            lambda k, il=il: w1t_cur[
                                :, k * TB + il * P : k * TB + (il + 1) * P
                            ],
                        )
                w1t_cur = w1t_next

            mm2_block(2)
            mm2_block(3, tail=True)

    nc.compile()
    return nc


def _get_nc():
    if "nc" not in _CACHE:
        _CACHE["nc"] = _build()
    return _CACHE["nc"]


def _prep_in_maps(x_sorted, w1, w2, expert_counts):
    counts = np.asarray(expert_counts, dtype=np.int64)
    n = x_sorted.shape[0]
    offsets = np.cumsum(counts)
    eid = np.searchsorted(offsets, np.arange(n), side="right")

    in_maps = []
    row_idx = []
    for e in range(E):
        rows = np.nonzero(eid == e)[0]
        assert len(rows) <= T, f"expert {e} overflows capacity {T}"
        xe = np.zeros((T, H), dtype=np.float32)
        xe[: len(rows)] = x_sorted[rows]
        row_idx.append(rows)

        # xg[p, b*HB*TB + k*TB + c] = xe[b*TB+c, k*P+p]
        xg = (
            xe.reshape(NB, TB, HB, P)
            .transpose(3, 0, 2, 1)
            .reshape(P, NB * HB * TB)
        )
        w1e = np.asarray(w1[e], dtype=np.float32)  # [H, I]
        # w1a[p, il*HB*P + k*P + c] = w1e[k*P+p, il*P+c]  (i-tiles 0..3)
        w1a = (
            w1e[:, : GI * P]
            .reshape(HB, P, GI, P)
            .transpose(1, 2, 0, 3)
            .reshape(P, GI * HB * P)
        )
        # w1b[p, g'*HB*TB + k*TB + c] = w1e[k*P+p, (g'+1)*TB+c]
        w1b = (
            w1e[:, TB:]
            .reshape(HB, P, NG - 1, TB)
            .transpose(1, 2, 0, 3)
            .reshape(P, (NG - 1) * HB * TB)
        )
        w2e = np.asarray(w2[e], dtype=np.float32)  # [I, H]
        # w2h[p, i*H + c] = w2e[i*P+p, c]
        w2h = w2e.reshape(IB, P, H).transpose(1, 0, 2).reshape(P, IB * H)

        in_maps.append(
            {
                "xg": np.ascontiguousarray(xg).astype(ml_dtypes.bfloat16),
                "w1a": np.ascontiguousarray(w1a).astype(ml_dtypes.bfloat16),
                "w1b": np.ascontiguousarray(w1b).astype(ml_dtypes.bfloat16),
                "w2h": np.ascontiguousarray(w2h).astype(ml_dtypes.bfloat16),
            }
        )
    return in_maps, row_idx


def kernel(x_sorted, w1, w2, expert_counts, local_expert_indices, **_unused):
    from concourse.bass_utils import run_bass_kernel_spmd

    x_sorted = np.ascontiguousarray(x_sorted, dtype=np.float32)
    nc = _get_nc()
    in_maps, row_idx = _prep_in_maps(x_sorted, w1, w2, expert_counts)
    res = run_bass_kernel_spmd(nc, in_maps, list(range(E))).results

    n = x_sorted.shape[0]
    out = np.zeros((n, H), dtype=np.float32)
    for e in range(E):
        rows = row_idx[e]
        out[rows] = np.asarray(res[e]["y"][: len(rows)], dtype=np.float32)
    return out


# revision 7
# speedup vs baseline: 1.0049x; 1.0032x over previous
"""MoE ExpertGroup kernel for Trainium2 (8 NeuronCores, expert-parallel).

Problem: E=8 experts, H=1024, I=4096, N=16384 tokens sorted by expert.
y[t] = gelu_tanh(x[t] @ w1[e(t)]) @ w2[e(t)]

Sharding: expert-parallel - core e holds expert e's weights and processes
expert e's token block (balanced routing: 2048 tokens/core). All matmul
operands are bf16 (full-rate on the PE, half the DMA traffic); y is
returned in bf16 and upcast on the host (rel err ~4e-3 vs 2e-2 budget).

Per-core structure (PE stream floor: 2048 matmuls x 512 cols @2.4GHz
= 442us; everything else is edges):
- Host pre-packs x/w1/w2 into k-major SBUF-shaped layouts so each DMA
  is ONE large contiguous-line transfer (the Sync queue issues triggers
  at only ~1.6/us, and each trigger fans out over all 16 DMA engines).
- MM1 pass 1 (blocks b0,b1): group g0 runs single-block chains (all b0,
  then all b1) with per-i-tile w1 triggers, so the first chain needs
  only 1.25MB landed (~12us) instead of 3MB (~18us). Groups g1..g7 run
  b0+b1 chains off one 1MB group tile, double buffered, prefetched one
  group ahead.
- MM2 accumulates all 32 I-tiles of a token tile into one PSUM group.
  y stored as bf16. For the very last token tile the second output half
  is drained by the Vector engine in parallel with the Scalar act to
  shorten the tail.
- 12 warmup matmuls ramp the PE clock (HAM un-throttle needs ~3.4us of
  busy) while the first DMAs land.
"""

import sys

sys.path.insert(0, "/opt/trn_rl_repo")

import numpy as np
import ml_dtypes

E = 8
H = 1024
I = 4096
N_TOK = 16384
T = N_TOK // E

P = 128
TB = 512
NB = T // TB          # 4 token blocks
HB = H // P           # 8 k-tiles (contraction for MM1)
IB = I // P           # 32 i-tiles
GI = 4                # i-tiles per w1 group
NG = IB // GI         # 8 groups
WARM = 11

_CACHE = {}


def _build():
    import concourse.bacc as bacc
    import concourse.mybir as mybir
    import concourse.tile as tile

    F32 = mybir.dt.float32
    BF16 = mybir.dt.bfloat16
    GELU = mybir.ActivationFunctionType.Gelu_apprx_tanh
    COPY = mybir.ActivationFunctionType.Copy

    nc = bacc.Bacc("TRN2", target_bir_lowering=False, debug=False, num_devices=E)

    # Host-packed layouts (see _prep_in_maps):
    #   xg [P, (b,k,c)]   : xg[p, b*HB*TB + k*TB + c] = x[b*TB+c, k*P+p]
    #   w1a [P, (il,k,c)] : w1a[p, il*HB*P + k*P + c] = w1[k*P+p, il*P+c]
    #   w1b [P, (g',k,c)] : w1b[p, g'*HB*TB + k*TB + c] = w1[k*P+p, (g'+1)*TB+c]
    #   w2h [P, (i,c)]    : w2h[p, i*H + c] = w2[i*P+p, c]
    xg = nc.dram_tensor("xg", [P, NB * HB * TB], BF16, kind="ExternalInput").ap()
    w1a = nc.dram_tensor("w1a", [P, GI * HB * P], BF16, kind="ExternalInput").ap()
    w1b = nc.dram_tensor(
        "w1b", [P, (NG - 1) * HB * TB], BF16, kind="ExternalInput"
    ).ap()
    w2h = nc.dram_tensor("w2h", [P, IB * H], BF16, kind="ExternalInput").ap()
    y = nc.dram_tensor("y", [T, H], BF16, kind="ExternalOutput").ap()

    with tile.TileContext(nc) as tc:
        with (
            tc.tile_pool(name="xp", bufs=1) as x_pool,
            tc.tile_pool(name="w1ap", bufs=1) as w1a_pool,
            tc.tile_pool(name="w1p", bufs=2) as w1_pool,
            tc.tile_pool(name="w2p", bufs=1) as w2_pool,
            tc.tile_pool(name="hp", bufs=1) as h_pool,
            tc.tile_pool(name="yp", bufs=4) as y_pool,
            tc.tile_pool(name="ph", bufs=4, space="PSUM") as ph_pool,
            tc.tile_pool(name="pyA", bufs=2, space="PSUM") as pyA_pool,
            tc.tile_pool(name="pyB", bufs=2, space="PSUM") as pyB_pool,
        ):
            # warmup first: PE busy while the first DMAs land (HAM ramp)
            warm = y_pool.tile([P, TB], BF16, tag="warm", name="warm")
            nc.vector.memset(warm[:], 0.0)
            for wi in range(WARM):
                pw = ph_pool.tile([P, TB], F32, tag="ph", name="pw")
                nc.tensor.matmul(pw[:], warm[:, :P], warm[:], start=True, stop=True)

            # resident tiles
            xt = [
                x_pool.tile([P, HB * TB], BF16, tag=f"x{b}", name=f"x{b}")
                for b in range(NB)
            ]
            w1at = [
                w1a_pool.tile([P, HB * P], BF16, tag=f"a{il}", name=f"a{il}")
                for il in range(GI)
            ]
            w2t = [
                w2_pool.tile([P, GI * H], BF16, tag=f"q{q}", name=f"q{q}")
                for q in range(IB // GI)
            ]

            def trig_x(b):
                nc.sync.dma_start(
                    out=xt[b][:], in_=xg[:, b * HB * TB : (b + 1) * HB * TB]
                )

            def trig_x_half(b, half):
                # k-halves land separately so MM1 chains can start mid-delivery
                o = b * HB * TB + half * (HB // 2) * TB
                w = (HB // 2) * TB
                nc.sync.dma_start(
                    out=xt[b][:, half * w : (half + 1) * w],
                    in_=xg[:, o : o + w],
                )

            def trig_w1a(il):
                nc.sync.dma_start(
                    out=w1at[il][:], in_=w1a[:, il * HB * P : (il + 1) * HB * P]
                )

            def trig_w1g(g):
                wt = w1_pool.tile([P, HB * TB], BF16, tag="wg", name="wg")
                nc.sync.dma_start(
                    out=wt[:], in_=w1b[:, (g - 1) * HB * TB : g * HB * TB]
                )
                return wt

            def trig_w1g0b():
                wt = w1a_pool.tile([P, GI * HB * P], BF16, tag="g0b", name="g0b")
                nc.sync.dma_start(out=wt[:], in_=w1a[:])
                return wt

            def trig_w2(q):
                nc.sync.dma_start(
                    out=w2t[q][:], in_=w2h[:, q * GI * H : (q + 1) * GI * H]
                )

            hT = {}

            def chain(b, i, lhsT_of_k):
                """One MM1 chain: hT[b%2, i] = gelu(w1[:, i-tile].T @ x[b])."""
                pht = ph_pool.tile([P, TB], F32, tag="ph", name="ph")
                for k in range(HB):
                    nc.tensor.matmul(
                        pht[:],
                        lhsT_of_k(k),
                        xt[b][:, k * TB : (k + 1) * TB],
                        start=(k == 0),
                        stop=(k == HB - 1),
                    )
                st = b % 2
                ht = h_pool.tile([P, TB], BF16, tag=f"h{st}_{i}", name=f"h{st}_{i}")
                nc.scalar.activation(ht[:], pht[:], GELU)
                hT[(st, i)] = ht

            def mm2_block(b, tail=False):
                st = b % 2
                for tc_ in range(TB // P):
                    pa = pyA_pool.tile([P, TB], F32, tag="pyA", name="pyA")
                    pb = pyB_pool.tile([P, TB], F32, tag="pyB", name="pyB")
                    for i in range(IB):
                        hs = hT[(st, i)][:, tc_ * P : (tc_ + 1) * P]
                        w2v = w2t[i // GI][:, (i % GI) * H : (i % GI + 1) * H]
                        nc.tensor.matmul(
                            pa[:], hs, w2v[:, : H // 2],
                            start=(i == 0), stop=(i == IB - 1),
                        )
                        nc.tensor.matmul(
                            pb[:], hs, w2v[:, H // 2 :],
                            start=(i == 0), stop=(i == IB - 1),
                        )
                    ysb = y_pool.tile([P, H], BF16, tag="yt", name="yt")
                    t0 = b * TB + tc_ * P
                    nc.scalar.activation(ysb[:, : H // 2], pa[:], COPY)
                    nc.sync.dma_start(
                        out=y[t0 : t0 + P, : H // 2], in_=ysb[:, : H // 2]
                    )
                    nc.scalar.activation(ysb[:, H // 2 :], pb[:], COPY)
                    nc.sync.dma_start(
                        out=y[t0 : t0 + P, H // 2 :], in_=ysb[:, H // 2 :]
                    )

            def mm2_block_tail(b):
                """Like mm2_block but the very last token tile splits its
                second output half into two 256-col PSUM chains so the
                final act+store after the last matmul is only 256 cols,
                drained by the Vector engine with the store issued from the
                Scalar DGE queue (both off the Sync queue's critical path)."""
                st = b % 2
                Q = H // 4
                for tc_ in range(TB // P):
                    last = tc_ == TB // P - 1
                    pa = pyA_pool.tile([P, TB], F32, tag="pyA", name="pyA")
                    pb = pyB_pool.tile([P, TB], F32, tag="pyB", name="pyB")
                    if last:
                        pc2 = ph_pool.tile([P, TB], F32, tag="ph", name="ph")
                    for i in range(IB):
                        hs = hT[(st, i)][:, tc_ * P : (tc_ + 1) * P]
                        w2v = w2t[i // GI][:, (i % GI) * H : (i % GI + 1) * H]
                        nc.tensor.matmul(
                            pa[:], hs, w2v[:, : H // 2],
                            start=(i == 0), stop=(i == IB - 1),
                        )
                        if last:
                            nc.tensor.matmul(
                                pb[:, :Q], hs, w2v[:, H // 2 : 3 * Q],
                                start=(i == 0), stop=(i == IB - 1),
                            )
                            nc.tensor.matmul(
                                pc2[:, :Q], hs, w2v[:, 3 * Q :],
                                start=(i == 0), stop=(i == IB - 1),
                            )
                        else:
                            nc.tensor.matmul(
                                pb[:], hs, w2v[:, H // 2 :],
                                start=(i == 0), stop=(i == IB - 1),
                            )
                    ysb = y_pool.tile([P, H], BF16, tag="yt", name="yt")
                    t0 = b * TB + tc_ * P
                    nc.scalar.activation(ysb[:, : H // 2], pa[:], COPY)
                    nc.sync.dma_start(
                        out=y[t0 : t0 + P, : H // 2], in_=ysb[:, : H // 2]
                    )
                    if last:
                        nc.vector.tensor_copy(ysb[:, H // 2 : 3 * Q], pb[:, :Q])
                        nc.sync.dma_start(
                            out=y[t0 : t0 + P, H // 2 : 3 * Q],
                            in_=ysb[:, H // 2 : 3 * Q],
                        )
                        nc.vector.tensor_copy(ysb[:, 3 * Q :], pc2[:, :Q])
                        nc.scalar.dma_start(
                            out=y[t0 : t0 + P, 3 * Q :], in_=ysb[:, 3 * Q :]
                        )
                    else:
                        nc.scalar.activation(ysb[:, H // 2 :], pb[:], COPY)
                        nc.sync.dma_start(
                            out=y[t0 : t0 + P, H // 2 :], in_=ysb[:, H // 2 :]
                        )

            # ---- prologue DMA triggers (order = landing order) ----
            trig_w1a(0)
            trig_x_half(0, 0)
            trig_x_half(0, 1)
            trig_w1a(1)
            trig_w1a(2)
            trig_w1a(3)
            trig_x_half(1, 0)
            trig_x_half(1, 1)
            w1t_cur = trig_w1g(1)

            # ---- pass 1 (blocks 0,1) ----
            # g0: single-block chains off the per-i-tile w1a tiles
            for b in (0, 1):
                for il in range(GI):
                    chain(b, il, lambda k, il=il: w1at[il][:, k * P : (k + 1) * P])
            # g1..g7: paired blocks off 1MB group tiles, prefetch g+1
            for g in range(1, NG):
                w1t_next = trig_w1g(g + 1) if g + 1 < NG else None
                if g == 2:
                    trig_w2(0)
                elif g == 3:
                    trig_w2(1)
                elif g == 4:
                    trig_w2(2)
                    trig_x(2)
                elif g == 5:
                    trig_w2(3)
                    trig_x(3)
                elif g == 6:
                    trig_w2(4)
                elif g == 7:
                    trig_w2(5)
                for b in (0, 1):
                    for il in range(GI):
                        i = g * GI + il
                        chain(
                            b,
                            i,
                            lambda k, il=il: w1t_cur[
                                :, k * TB + il * P : k * TB + (il + 1) * P
                            ],
                        )
                w1t_cur = w1t_next

            # prefetch pass-2 w1 (g0 packed tile + group 1) before the y DMAs
            # of mm2 enter the Sync queue
            w1g0b = trig_w1g0b()
            w1t_cur = trig_w1g(1)
            trig_w2(6)
            trig_w2(7)

            mm2_block(0)
            mm2_block(1)

            # ---- pass 2 (blocks 2,3) ----
            for b in (2, 3):
                for il in range(GI):
                    chain(
                        b,
                        il,
                        lambda k, il=il: w1g0b[
                            :, il * HB * P + k * P : il * HB * P + (k + 1) * P
                        ],
                    )
            for g in range(1, NG):
                w1t_next = trig_w1g(g + 1) if g + 1 < NG else None
                for b in (2, 3):
                    for il in range(GI):
                        i = g * GI + il
                        chain(
                            b,
                            i,
                            lambda k, il=il: w1t_cur[
                                :, k * TB + il * P : k * TB + (il + 1) * P
                            ],
                        )
                w1t_cur = w1t_next

            mm2_block(2)
            mm2_block_tail(3)

    nc.compile()
    return nc


def _get_nc():
    if "nc" not in _CACHE:
        _CACHE["nc"] = _build()
    return _CACHE["nc"]


def _prep_in_maps(x_sorted, w1, w2, expert_counts):
    counts = np.asarray(expert_counts, dtype=np.int64)
    n = x_sorted.shape[0]
    offsets = np.cumsum(counts)
    eid = np.searchsorted(offsets, np.arange(n), side="right")

    in_maps = []
    row_idx = []
    for e in range(E):
        rows = np.nonzero(eid == e)[0]
        assert len(rows) <= T, f"expert {e} overflows capacity {T}"
        xe = np.zeros((T, H), dtype=np.float32)
        xe[: len(rows)] = x_sorted[rows]
        row_idx.append(rows)

        # xg[p, b*HB*TB + k*TB + c] = xe[b*TB+c, k*P+p]
        xg = (
            xe.reshape(NB, TB, HB, P)
            .transpose(3, 0, 2, 1)
            .reshape(P, NB * HB * TB)
        )
        w1e = np.asarray(w1[e], dtype=np.float32)  # [H, I]
        # w1a[p, il*HB*P + k*P + c] = w1e[k*P+p, il*P+c]  (i-tiles 0..3)
        w1a = (
            w1e[:, : GI * P]
            .reshape(HB, P, GI, P)
            .transpose(1, 2, 0, 3)
            .reshape(P, GI * HB * P)
        )
        # w1b[p, g'*HB*TB + k*TB + c] = w1e[k*P+p, (g'+1)*TB+c]
        w1b = (
            w1e[:, TB:]
            .reshape(HB, P, NG - 1, TB)
            .transpose(1, 2, 0, 3)
            .reshape(P, (NG - 1) * HB * TB)
        )
        w2e = np.asarray(w2[e], dtype=np.float32)  # [I, H]
        # w2h[p, i*H + c] = w2e[i*P+p, c]
        w2h = w2e.reshape(IB, P, H).transpose(1, 0, 2).reshape(P, IB * H)

        in_maps.append(
            {
                "xg": np.ascontiguousarray(xg).astype(ml_dtypes.bfloat16),
                "w1a": np.ascontiguousarray(w1a).astype(ml_dtypes.bfloat16),
                "w1b": np.ascontiguousarray(w1b).astype(ml_dtypes.bfloat16),
                "w2h": np.ascontiguousarray(w2h).astype(ml_dtypes.bfloat16),
            }
        )
    return in_maps, row_idx


def kernel(x_sorted, w1, w2, expert_counts, local_expert_indices, **_unused):
    from concourse.bass_utils import run_bass_kernel_spmd

    x_sorted = np.ascontiguousarray(x_sorted, dtype=np.float32)
    nc = _get_nc()
    in_maps, row_idx = _prep_in_maps(x_sorted, w1, w2, expert_counts)
    res = run_bass_kernel_spmd(nc, in_maps, list(range(E))).results

    n = x_sorted.shape[0]
    out = np.zeros((n, H), dtype=np.float32)
    for e in range(E):
        rows = row_idx[e]
        out[rows] = np.asarray(res[e]["y"][: len(rows)], dtype=np.float32)
    return out
